# revision 1
# baseline (speedup 1.0000x reference)
"""GCN aggregator kernel for Trainium2 (Bass/Tile), 8-core data-parallel.

Computes: out = relu(((sum_k neigh[:,k,:] + self) / (K+1)) @ W + b)
Sharding: nodes (N) split evenly across 8 NeuronCores; W/b replicated.

Per 128-node tile on each core:
  1. DMA neigh tile [128, K*D] + self tile [128, D]           (sync HWDGE)
  2. DVE reduce_sum over k (strided AP) + add self            (VectorE)
  3. PE transpose sum -> sumT in PSUM, ACT copy w/ 1/(K+1)    (TensorE/ScalarE)
  4. PE GEMM sumT.T @ W accumulated over 4 d-chunks + bias    (TensorE)
  5. ACT relu PSUM->SBUF, DMA store                           (ScalarE HWDGE)
"""

import os
import sys

import numpy as np

for _p in ("/opt/trn_rl_repo", "/root/.axon_site/_ro/trn_rl_repo"):
    if os.path.isdir(_p) and _p not in sys.path:
        sys.path.insert(0, _p)

import concourse.bass as bass
import concourse.tile as tile
from concourse import bacc, mybir
from concourse.masks import make_identity

N, K, D, O = 16384, 25, 512, 1024
N_CORES = 8
P = 128  # nodes per tile (partition count)
INV = 1.0 / (K + 1)
FP = mybir.dt.float32


def _tree_fold(nc, t, g):
    """In-place pairwise fold of `g` contiguous D-sized groups in tile t;
    result lands in t[:, :D]."""
    while g > 1:
        lo = g // 2
        nc.vector.tensor_add(
            t[:, : lo * D], t[:, : lo * D], t[:, (g - lo) * D : g * D]
        )
        g -= lo


def build_nc(n_nodes: int, neigh_bufs: int = 3) -> bass.Bass:
    """Build the per-core Bass program for a shard of `n_nodes` nodes."""
    assert n_nodes % P == 0
    nt = n_nodes // P

    nc = bacc.Bacc("TRN2", target_bir_lowering=False, debug=False)
    self_h = nc.dram_tensor("self_vecs", [n_nodes, D], FP, kind="ExternalInput")
    neigh_h = nc.dram_tensor("neigh_vecs", [n_nodes, K, D], FP, kind="ExternalInput")
    w_h = nc.dram_tensor("W", [D, O], FP, kind="ExternalInput")
    b_h = nc.dram_tensor("b", [O], FP, kind="ExternalInput")
    out_h = nc.dram_tensor("out", [n_nodes, O], FP, kind="ExternalOutput")

    n_dc = D // P  # d-chunks for the GEMM contraction

    with tile.TileContext(nc) as tc:
        with (
            tc.tile_pool(name="const", bufs=1) as const_pool,
            tc.tile_pool(name="neigh", bufs=neigh_bufs) as neigh_pool,
            tc.tile_pool(name="small", bufs=3) as small_pool,
            tc.tile_pool(name="outp", bufs=3) as out_pool,
            tc.tile_pool(name="ps_t", bufs=2, space="PSUM") as ps_t_pool,
            tc.tile_pool(name="ps_o", bufs=2, space="PSUM") as ps_o_pool,
        ):
            # --- constants (w_sb/b_sb DMAs are emitted after tile 0's loads
            # below, so the neigh stream starts immediately on the ring; W is
            # only needed by the first GEMM at ~20us) ---
            # w_sb[p, c, o] = W[c*128 + p, o] -> chunk c is the rhs for d-chunk c
            w_sb = const_pool.tile([P, n_dc * O], FP)
            b_sb = const_pool.tile([1, O], FP)
            ident = const_pool.tile([P, P], FP)
            make_identity(nc, ident)
            ones = const_pool.tile([1, P], FP)
            nc.gpsimd.memset(ones, 1.0)

            def transpose_scaled(src):
                """PE-transpose src [n,d] into [d,n] chunks, scale by 1/(K+1)
                on the PSUM->SBUF copy."""
                tps = ps_t_pool.tile([P, D], FP, tag="tps", name="tps")
                for c in range(n_dc):
                    nc.tensor.transpose(
                        tps[:, bass.ts(c, P)], src[:, bass.ts(c, P)], ident
                    )
                t_sb = small_pool.tile([P, D], FP, tag="tsb", name="tsb")
                nc.scalar.activation(
                    t_sb, tps, mybir.ActivationFunctionType.Copy, scale=INV
                )
                return t_sb

            def gemm_acc(out_pss, sumT, start):
                for c in range(n_dc):
                    for oh in range(len(out_pss)):
                        nc.tensor.matmul(
                            out_pss[oh],
                            lhsT=sumT[:, bass.ts(c, P)],
                            rhs=w_sb[:, c * O + oh * 512 : c * O + oh * 512 + 512],
                            start=(start and c == 0),
                            stop=False,
                        )

            k1n = K // 2  # 12 neigh groups in half 1 (+ self = 13 groups)
            k2 = K - k1n  # 13 neigh groups in half 2
            for i in range(nt):
                # split the neigh load so the k-sum (DVE tree adds; these run
                # at model speed where tensor_reduce measured ~1.6x slower)
                # starts while the second half streams, and SBUF slots
                # release at half-tile granularity. self_vecs rides in half 1
                # as a 13th group so no separate add is needed.
                nh1 = neigh_pool.tile([P, (k1n + 1) * D], FP, tag="nh1", name="nh1")
                nc.sync.dma_start(nh1[:, : k1n * D], neigh_h[bass.ts(i, P), 0:k1n, :])
                nc.sync.dma_start(nh1[:, k1n * D :], self_h[bass.ts(i, P), :])
                nh2 = neigh_pool.tile([P, k2 * D], FP, tag="nh2", name="nh2")
                nc.sync.dma_start(nh2, neigh_h[bass.ts(i, P), k1n:K, :])
                if i == 0:
                    nc.sync.dma_start(
                        w_sb, w_h[:, :].rearrange("(c p) o -> p c o", p=P)
                    )
                    nc.sync.dma_start(b_sb, b_h[:])
                n_oh = O // 512

                def make_out_pss():
                    return [
                        ps_o_pool.tile(
                            [P, 512], FP, tag=f"out_ps{oh}", name=f"out_ps{oh}"
                        )
                        for oh in range(n_oh)
                    ]

                _tree_fold(nc, nh1, k1n + 1)
                _tree_fold(nc, nh2, k2)
                summ = small_pool.tile([P, D], FP)
                nc.vector.tensor_add(summ, nh1[:, :D], nh2[:, :D])
                sumT = transpose_scaled(summ)
                out_sb = out_pool.tile([P, O], FP)
                out_pss = make_out_pss()
                gemm_acc(out_pss, sumT, start=True)

                for oh in range(n_oh):
                    # bias via K=1 matmul: ones.T @ b broadcasts b over nodes
                    nc.tensor.matmul(
                        out_pss[oh],
                        lhsT=ones,
                        rhs=b_sb[:, bass.ts(oh, 512)],
                        start=False,
                        stop=True,
                    )
                    nc.scalar.activation(
                        out_sb[:, bass.ts(oh, 512)],
                        out_pss[oh],
                        mybir.ActivationFunctionType.Relu,
                    )
                nc.scalar.dma_start(out_h[bass.ts(i, P), :], out_sb)

    nc.compile()
    return nc


def shard_inputs(inputs: dict) -> list[dict]:
    n = inputs["self_vecs"].shape[0]
    per = n // N_CORES
    maps = []
    for c in range(N_CORES):
        sl = slice(c * per, (c + 1) * per)
        maps.append(
            {
                "self_vecs": np.ascontiguousarray(inputs["self_vecs"][sl], np.float32),
                "neigh_vecs": np.ascontiguousarray(
                    inputs["neigh_vecs"][sl], np.float32
                ),
                "W": np.ascontiguousarray(inputs["W"], np.float32),
                "b": np.ascontiguousarray(inputs["b"], np.float32),
            }
        )
    return maps


def run_sharded(inputs: dict, trace: bool = False, **kwargs):
    from concourse.bass_utils import run_bass_kernel_spmd

    in_maps = shard_inputs(inputs)
    n_nodes = in_maps[0]["self_vecs"].shape[0]
    nc = build_nc(n_nodes)
    res = run_bass_kernel_spmd(
        nc, in_maps, core_ids=list(range(N_CORES)), trace=trace, **kwargs
    )
    out = np.concatenate([res.results[c]["out"] for c in range(N_CORES)], axis=0)
    return out, res


def kernel(**inputs) -> np.ndarray:
    out, _ = run_sharded(inputs, trace=False)
    return out



# revision 3
# speedup vs baseline: 2.2935x; 2.2935x over previous
"""GCN aggregator kernel for Trainium2 (Bass/Tile), 8-core data-parallel.

Computes: out = relu(((sum_k neigh[:,k,:] + self) / (K+1)) @ W + b)
Sharding: nodes (N) split evenly across 8 NeuronCores; W/b replicated.

The kernel is HBM-bandwidth bound (ridge regime): per-core traffic in f32
is ~119MB against a ~358 GB/s per-NC cap. The rel-err budget (2e-2) has
~4x margin over bf16 rounding (~5e-3 measured end-to-end), so all streams
are cast to bf16 on the host, halving HBM traffic and doubling DVE
throughput (2x perf mode).

Per 128-node tile on each core:
  1. DMA packed [neigh;self] tile in two 13-group halves    (sync HWDGE)
  2. DVE pairwise tree-fold of each half + final add        (VectorE 2x)
  3. PE transpose sum -> PSUM, ACT copy w/ 1/(K+1) -> bf16  (TensorE/ScalarE)
  4. PE GEMM sumT.T @ W (bf16) accumulated over 4 d-chunks
     + bias via ones-matmul                                 (TensorE)
  5. ACT relu PSUM->SBUF bf16, DMA store                    (ScalarE HWDGE)

Host: inputs packed+cast to bf16 (numpy round-to-nearest via ml_dtypes);
bf16 output upcast to f32 before returning.
"""

import os
import sys

import numpy as np
import ml_dtypes

for _p in ("/opt/trn_rl_repo", "/root/.axon_site/_ro/trn_rl_repo"):
    if os.path.isdir(_p) and _p not in sys.path:
        sys.path.insert(0, _p)

import concourse.bass as bass
import concourse.tile as tile
from concourse import bacc, mybir
from concourse.masks import make_identity

N, K, D, O = 16384, 25, 512, 1024
G = K + 1  # neigh groups + self
N_CORES = 8
P = 128  # nodes per tile (partition count)
INV = 1.0 / (K + 1)
FP = mybir.dt.float32
BF = mybir.dt.bfloat16
NP_BF = ml_dtypes.bfloat16


def _tree_fold(nc, t, g):
    """In-place pairwise fold of `g` contiguous D-sized groups in tile t;
    result lands in t[:, :D]."""
    while g > 1:
        lo = g // 2
        nc.vector.tensor_add(
            t[:, : lo * D], t[:, : lo * D], t[:, (g - lo) * D : g * D]
        )
        g -= lo


def build_nc(n_nodes: int, neigh_bufs: int = 6) -> bass.Bass:
    """Build the per-core Bass program for a shard of `n_nodes` nodes."""
    assert n_nodes % P == 0
    nt = n_nodes // P

    nc = bacc.Bacc("TRN2", target_bir_lowering=False, debug=False)
    packed_h = nc.dram_tensor("packed", [n_nodes, G * D], BF, kind="ExternalInput")
    w_h = nc.dram_tensor("W", [D, O], BF, kind="ExternalInput")
    b_h = nc.dram_tensor("b", [O], BF, kind="ExternalInput")
    out_h = nc.dram_tensor("out", [n_nodes, O], BF, kind="ExternalOutput")

    n_dc = D // P  # d-chunks for the GEMM contraction
    g1 = G // 2  # 13 groups per half
    g2 = G - g1

    with tile.TileContext(nc) as tc:
        with (
            tc.tile_pool(name="const", bufs=1) as const_pool,
            tc.tile_pool(name="neigh", bufs=neigh_bufs) as neigh_pool,
            tc.tile_pool(name="small", bufs=3) as small_pool,
            tc.tile_pool(name="outp", bufs=3) as out_pool,
            tc.tile_pool(name="ps_t", bufs=2, space="PSUM") as ps_t_pool,
            tc.tile_pool(name="ps_o", bufs=2, space="PSUM") as ps_o_pool,
        ):
            # --- constants (w_sb/b_sb DMAs are emitted after tile 0's loads
            # below, so the neigh stream starts immediately on the ring; W is
            # only needed by the first GEMM) ---
            # w_sb[p, c, o] = W[c*128 + p, o] -> chunk c is the rhs for d-chunk c
            w_sb = const_pool.tile([P, n_dc * O], BF)
            b_sb = const_pool.tile([1, O], BF)
            ident = const_pool.tile([P, P], BF)
            make_identity(nc, ident)
            ones = const_pool.tile([1, P], BF)
            nc.gpsimd.memset(ones, 1.0)

            def transpose_scaled(src):
                """PE-transpose src [n,d] into [d,n] chunks, scale by 1/(K+1)
                on the PSUM->SBUF copy (cast to bf16 for the GEMM lhsT)."""
                tps = ps_t_pool.tile([P, D], BF, tag="tps", name="tps")
                for c in range(n_dc):
                    nc.tensor.transpose(
                        tps[:, bass.ts(c, P)], src[:, bass.ts(c, P)], ident
                    )
                t_sb = small_pool.tile([P, D], BF, tag="tsb", name="tsb")
                nc.scalar.activation(
                    t_sb, tps, mybir.ActivationFunctionType.Copy, scale=INV
                )
                return t_sb

            n_oh = O // 512

            def gemm_acc(out_pss, sumT, start):
                for c in range(n_dc):
                    for oh in range(len(out_pss)):
                        nc.tensor.matmul(
                            out_pss[oh],
                            lhsT=sumT[:, bass.ts(c, P)],
                            rhs=w_sb[:, c * O + oh * 512 : c * O + oh * 512 + 512],
                            start=(start and c == 0),
                            stop=False,
                        )

            for i in range(nt):
                # split the load in halves so the k-sum (DVE tree adds in
                # bf16 2x mode) starts while the second half streams, and
                # SBUF slots release at half-tile granularity. self_vecs is
                # pre-packed as the 26th group so no separate add is needed.
                nh1 = neigh_pool.tile([P, g1 * D], BF, tag="nh1", name="nh1")
                nc.sync.dma_start(nh1, packed_h[bass.ts(i, P), : g1 * D])
                nh2 = neigh_pool.tile([P, g2 * D], BF, tag="nh2", name="nh2")
                nc.sync.dma_start(nh2, packed_h[bass.ts(i, P), g1 * D :])
                if i == 0:
                    nc.sync.dma_start(
                        w_sb, w_h[:, :].rearrange("(c p) o -> p c o", p=P)
                    )
                    nc.sync.dma_start(b_sb, b_h[:])

                def make_out_pss():
                    return [
                        ps_o_pool.tile(
                            [P, 512], FP, tag=f"out_ps{oh}", name=f"out_ps{oh}"
                        )
                        for oh in range(n_oh)
                    ]

                _tree_fold(nc, nh1, g1)
                _tree_fold(nc, nh2, g2)
                summ = small_pool.tile([P, D], BF)
                nc.vector.tensor_add(summ, nh1[:, :D], nh2[:, :D])
                sumT = transpose_scaled(summ)
                out_sb = out_pool.tile([P, O], BF)
                out_pss = make_out_pss()
                gemm_acc(out_pss, sumT, start=True)

                for oh in range(n_oh):
                    # bias via K=1 matmul: ones.T @ b broadcasts b over nodes
                    nc.tensor.matmul(
                        out_pss[oh],
                        lhsT=ones,
                        rhs=b_sb[:, bass.ts(oh, 512)],
                        start=False,
                        stop=True,
                    )
                    nc.scalar.activation(
                        out_sb[:, bass.ts(oh, 512)],
                        out_pss[oh],
                        mybir.ActivationFunctionType.Relu,
                    )
                nc.scalar.dma_start(out_h[bass.ts(i, P), :], out_sb)

    nc.compile()
    return nc


def shard_inputs(inputs: dict) -> list[dict]:
    n = inputs["self_vecs"].shape[0]
    per = n // N_CORES
    # pack [neigh ; self] into one contiguous bf16 stream: [N, G, D]
    packed = np.empty((n, G, D), dtype=NP_BF)
    packed[:, :K] = inputs["neigh_vecs"]
    packed[:, K] = inputs["self_vecs"]
    packed = packed.reshape(n, G * D)
    w_bf = np.ascontiguousarray(inputs["W"], dtype=NP_BF)
    b_bf = np.ascontiguousarray(inputs["b"], dtype=NP_BF)
    maps = []
    for c in range(N_CORES):
        sl = slice(c * per, (c + 1) * per)
        maps.append({"packed": packed[sl], "W": w_bf, "b": b_bf})
    return maps


def run_sharded(inputs: dict, trace: bool = False, **kwargs):
    from concourse.bass_utils import run_bass_kernel_spmd

    in_maps = shard_inputs(inputs)
    n_nodes = in_maps[0]["packed"].shape[0]
    nc = build_nc(n_nodes)
    res = run_bass_kernel_spmd(
        nc, in_maps, core_ids=list(range(N_CORES)), trace=trace, **kwargs
    )
    out = np.concatenate(
        [res.results[c]["out"] for c in range(N_CORES)], axis=0
    ).astype(np.float32)
    return out, res


def kernel(**inputs) -> np.ndarray:
    out, _ = run_sharded(inputs, trace=False)
    return out
